# revision 20
# baseline (speedup 1.0000x reference)
"""Multi-head causal attention (B=4, S=2048, D=1024, H=16) on 8 TRN2 NeuronCores.

Sharding: core c handles batch b = c//2 and head-group hg = c%2 (8 heads each).
Each core computes Q/K/V projections for its (batch, head-group), causal
attention, and a partial output projection over its 512 head-dims.  The host
sums the two partials per batch and adds b_o.  No collectives.

Device-side layout choices:
  - x is passed transposed (xT [D, S]), loaded once into SBUF and reused by
    all projection chains.
  - Q and K are produced transposed (QT/KT [dq, S]); scores are computed
    transposed (S^T [kpos, q]).  K^T tiles keep the natural head-pair packing
    (head 2d in rows 0-63, head 2d+1 in rows 64-127) and the two heads'
    score matmuls run CONCURRENTLY on the PE via row tiling (64-row
    contraction each, tile_position (0,0) / (64,0)).
  - Scores for one kt tile and both heads land in one 2-bank [128, 1024]
    PSUM tile, so a single ACT exp covers both heads (halves ACT instruction
    overhead, the co-bottleneck).
  - No max-subtraction in softmax: scaled scores are ~N(0,1), exp is safe.
  - AV matmuls trim the below-diagonal (always-zero) query range instead of
    memsetting P; only the 128-wide diagonal square needs affine_select.
  - U (64 value dims + 64 denominator-broadcast rows, via ones columns in V)
    is copied out of PSUM to SBUF in fp16 right away so only 2 PSUM banks of
    accumulators are needed; 1/l = exp(-ln(l)) runs on the SBUF copy off the
    critical path (ln and exp share one ACT table set -> no table reloads).
"""

import sys
import os

sys.path.insert(0, "/opt/trn_rl_repo")

import numpy as np

import concourse.bacc as bacc
import concourse.mybir as mybir
import concourse.tile as tile
from concourse.bass_utils import run_bass_kernel_spmd

# The ACT table-load pass resolves each activation to the first table set
# containing it, which puts Exp (exp_and_others) and Ln
# (natural_log_exp_and_others) in different sets and reloads tables at every
# softmax normalization.  Restrict Exp/Ln to the one set that holds both so
# the whole kernel runs off a single table load.
_orig_get_tables = bacc.get_activation_tables


def _patched_tables(arch):
    t = _orig_get_tables(arch)
    for name, fns in t.items():
        if name != "natural_log_exp_and_others":
            fns.discard(mybir.ActivationFunctionType.Exp)
            fns.discard(mybir.ActivationFunctionType.Ln)
    return t


bacc.get_activation_tables = _patched_tables

B, S, D, H = 4, 2048, 1024, 16
DK = D // H          # 64
HH = H // 2          # 8 heads per core
HD = HH * DK         # 512 head-dims per core
N_CORES = 8

F32 = mybir.dt.float32
F16 = mybir.dt.float16

SCALE = 1.0 / np.sqrt(DK)


def build_nc(s=S):
    """Build the per-core SPMD program.  `s` is the sequence length (tunable
    for small-scale simulation; must be a multiple of 1024)."""
    assert s % 1024 == 0
    n_qb = s // 512          # 512-wide q blocks
    n_t128 = s // 128        # 128-wide token tiles
    n_tbb = s // 1024        # 1024-wide token blocks (projection chains)
    n_dt = D // 128          # din tiles (8)

    nc = bacc.Bacc("TRN2", target_bir_lowering=False, debug=False,
                   num_devices=N_CORES)

    xT = nc.dram_tensor("xT", [D, s], F16, kind="ExternalInput")
    wqT = nc.dram_tensor("wqT", [D, HD], F16, kind="ExternalInput")
    wkT = nc.dram_tensor("wkT", [D, HD], F16, kind="ExternalInput")
    wvT = nc.dram_tensor("wvT", [D, HD], F16, kind="ExternalInput")
    woT = nc.dram_tensor("woT", [HD, D], F16, kind="ExternalInput")
    out = nc.dram_tensor("out", [s, D], F16, kind="ExternalOutput")

    with tile.TileContext(nc) as tc:
        with tc.tile_pool(name="persist", bufs=1) as persist, \
             tc.tile_pool(name="pT", bufs=20) as pT_pool, \
             tc.tile_pool(name="usb", bufs=3) as usb_pool, \
             tc.tile_pool(name="rb", bufs=2) as rb_pool, \
             tc.tile_pool(name="aoT", bufs=12) as aoT_pool, \
             tc.tile_pool(name="outsb", bufs=2) as out_pool, \
             tc.tile_pool(name="spsum", bufs=2, space="PSUM") as spsum, \
             tc.tile_pool(name="chpsum", bufs=1, space="PSUM") as chpsum, \
             tc.tile_pool(name="upsum", bufs=1, space="PSUM") as upsum:

            # Persistent SBUF arrays (live for the whole kernel).
            # Q^T / K^T per head-pair: head 2d in rows 0-63, 2d+1 in 64-127.
            qt_sb = [persist.tile([128, s], F16, tag=f"qt{d}", name=f"qt{d}") for d in range(HD // 128)]
            kt_sb = [persist.tile([128, s], F16, tag=f"kt{d}", name=f"kt{d}") for d in range(HD // 128)]
            # V tiles hold [t, head, 2*dk]: cols 0-63 are V, cols 64-127 are
            # 1.0.  As the AV stationary this makes the matmul emit U^T on
            # psum rows 0-63 and the softmax denominator on rows 64-127.
            v_sb = [persist.tile([128, HH, 2 * DK], F16, tag=f"v{t}", name=f"v{t}") for t in range(n_t128)]
            wo_sb = [persist.tile([128, D], F16, tag=f"wo{d}", name=f"wo{d}") for d in range(HD // 128)]
            wv_sb = [persist.tile([128, HD], F16, tag=f"wv{i}", name=f"wv{i}") for i in range(n_dt)]
            # x resident: [tbb][i] -> [128, 1024] (din tile i, token block tbb)
            x_sb = [[persist.tile([128, 1024], F16, tag=f"x{tbb}_{i}", name=f"x{tbb}_{i}")
                     for i in range(n_dt)] for tbb in range(n_tbb)]
            # Q/K weights in [128, 256] column slices (dq pairs) so the first
            # chain is gated on only 8 small DMAs.
            w_tiles = {}
            for wkey in ("q", "k"):
                for i in range(n_dt):
                    for dqp in range(HD // 256):
                        w_tiles[(wkey, i, dqp)] = persist.tile(
                            [128, 256], F16, tag=f"w{wkey}{i}_{dqp}",
                            name=f"w{wkey}{i}_{dqp}")

            # Ones columns of the V tiles (written once; V chains only write
            # cols 0-63).  On gpsimd to keep the DVE free.
            for t in range(n_t128):
                nc.gpsimd.memset(v_sb[t][:, :, DK:2 * DK], 1.0)

            # Warmup matmuls on scratch data: keep the PE busy through the
            # initial DMA lead-in so the HAM clock-gate is warm (2.4 GHz)
            # when real work starts.  Scratch memset on the (idle) DVE so
            # the warmup isn't queued behind the gpsimd memsets.
            scratch = persist.tile([128, 512], F16, tag="scratch", name="scratch")
            nc.vector.memset(scratch[:], 0.0)
            wm = spsum.tile([128, 1024], F32, tag="sp", name="wm")
            for _ in range(28):
                nc.tensor.matmul(wm[:, 0:512], lhsT=scratch[:, 0:128],
                                 rhs=scratch[:], start=True, stop=True)

            # DMAs in first-use order for the descending-block schedule:
            # k(dq0,tbb0) gates the first chain, then q(dq0,tbb1) needs wq
            # dqp0 + x tbb1, then wv (V chains start during the priming
            # pair), then the dq2-3 weights, then wo (out-proj is late).
            for i in range(n_dt):
                nc.sync.dma_start(
                    out=w_tiles[("k", i, 0)][:],
                    in_=wkT[i * 128:(i + 1) * 128, 0:256])
                nc.sync.dma_start(
                    out=x_sb[0][i][:], in_=xT[i * 128:(i + 1) * 128, 0:1024])
            for i in range(n_dt):
                nc.sync.dma_start(
                    out=w_tiles[("q", i, 0)][:],
                    in_=wqT[i * 128:(i + 1) * 128, 0:256])
                if n_tbb > 1:
                    nc.sync.dma_start(
                        out=x_sb[1][i][:],
                        in_=xT[i * 128:(i + 1) * 128, 1024:2048])
            for i in range(n_dt):
                nc.sync.dma_start(out=wv_sb[i][:], in_=wvT[i * 128:(i + 1) * 128, :])
            for dqp in range(1, HD // 256):
                for wkey, wdram in (("k", wkT), ("q", wqT)):
                    for i in range(n_dt):
                        nc.sync.dma_start(
                            out=w_tiles[(wkey, i, dqp)][:],
                            in_=wdram[i * 128:(i + 1) * 128,
                                      dqp * 256:(dqp + 1) * 256])
            for d in range(HD // 128):
                nc.sync.dma_start(out=wo_sb[d][:], in_=woT[d * 128:(d + 1) * 128, :])

            def emit_qk_chain(tbb, dq, is_k, pool=None):
                """Q^T or K^T projection for one 1024-token block and one
                head-pair dq, off the resident x tiles."""
                wkey = "k" if is_k else "q"
                pool = pool if pool is not None else chpsum
                ps = pool.tile([128, 1024], F32,
                               tag="sp" if pool is spsum else "ch", name="qk")
                # i outer / half inner: consecutive matmuls share the same
                # stationary weights.
                for i in range(n_dt):
                    w = w_tiles[(wkey, i, dq // 2)][:, (dq % 2) * 128:(dq % 2 + 1) * 128]
                    for half in range(2):
                        nc.tensor.matmul(
                            ps[:, half * 512:(half + 1) * 512],
                            lhsT=w,
                            rhs=x_sb[tbb][i][:, half * 512:(half + 1) * 512],
                            start=(i == 0), stop=(i == n_dt - 1),
                        )
                dst = kt_sb[dq] if is_k else qt_sb[dq]
                nc.vector.tensor_copy(
                    out=dst[:, tbb * 1024:(tbb + 1) * 1024], in_=ps[:])

            def emit_v_chain(tb):
                """V projection for one 128-token tile, spliced into the
                attention stream just before the q-block that needs it."""
                vp = chpsum.tile([128, 1024], F32, tag="ch", name="vp")
                for i in range(n_dt):
                    xs = x_sb[tb // 8][i][:, (tb % 8) * 128:(tb % 8 + 1) * 128]
                    nc.tensor.matmul(
                        vp[:, 0:512], lhsT=xs, rhs=wv_sb[i][:],
                        start=(i == 0), stop=(i == n_dt - 1),
                    )
                nc.vector.tensor_copy(
                    out=v_sb[tb][:, :, 0:DK],
                    in_=vp[:, 0:512].rearrange("p (h k) -> p h k", h=HH))

            def emit_score_kt(qb, hp, kt, pT):
                """Scores + exp for one kt tile, BOTH heads of the pair:
                row-tiled matmuls (64-contraction each) into one 2-bank psum
                tile, one exp over both."""
                lo = max(kt - 4 * qb, 0) * 128
                sp = spsum.tile([128, 1024], F32, tag="sp", name="sp")
                for hh in (0, 1):
                    nc.tensor.matmul(
                        sp[:, hh * 512 + lo:(hh + 1) * 512],
                        lhsT=kt_sb[hp][hh * 64:(hh + 1) * 64,
                                       kt * 128:(kt + 1) * 128],
                        rhs=qt_sb[hp][hh * 64:(hh + 1) * 64,
                                      qb * 512 + lo:(qb + 1) * 512],
                        start=True, stop=True,
                    )
                p = pT_pool.tile([128, 1024], F16, tag="p", name="p")
                nc.scalar.activation(
                    out=p[:, lo:1024], in_=sp[:, lo:1024],
                    func=mybir.ActivationFunctionType.Exp,
                    scale=float(SCALE))
                if kt >= 4 * qb:
                    # zero strict-upper (kpos > q) region of the 128-wide
                    # diagonal square, per head
                    for hh in (0, 1):
                        nc.gpsimd.affine_select(
                            out=p[:, hh * 512 + lo:hh * 512 + lo + 128],
                            in_=p[:, hh * 512 + lo:hh * 512 + lo + 128],
                            compare_op=mybir.AluOpType.is_ge,
                            fill=0.0, base=0, channel_multiplier=-1,
                            pattern=[[1, 128]])
                pT[kt] = (p, lo)

            def emit_pair(cur, nxt, pT_cur, pT_next, extra=None):
                """Interleave next pair's scores with current pair's AV
                chains at kt granularity: the PE gets AV matmuls to run
                while the ACT engine works through the scores' exps.
                `extra` maps a kt step to closures (projection/V chains)
                spliced in at that step."""
                nkt_cur = 4 * cur[0] + 4 if cur else 0
                nkt_nxt = 4 * nxt[0] + 4 if nxt else 0
                u = {}
                if cur:
                    ut = upsum.tile([128, 1024], F32, tag="u", name="ut")
                    for hh in (0, 1):
                        u[hh] = ut[:, hh * 512:(hh + 1) * 512]
                nsteps = max(nkt_cur, nkt_nxt)
                for kt in range(nsteps):
                    for work in (extra or {}).get(kt, ()):
                        work()
                    if kt < nkt_nxt:
                        emit_score_kt(nxt[0], nxt[1], kt, pT_next)
                    if kt < nkt_cur:
                        p, lo = pT_cur[kt]
                        for hh in (0, 1):
                            nc.tensor.matmul(
                                u[hh][:, lo:512],
                                lhsT=v_sb[kt][:, 2 * cur[1] + hh, :],
                                rhs=p[:, hh * 512 + lo:(hh + 1) * 512],
                                start=(kt == 0), stop=(kt == nkt_cur - 1),
                            )
                for work in (extra or {}).get(nsteps, ()):
                    work()
                if not cur:
                    return None
                # Evacuate U to SBUF fp16 (rows 64-127: PSUM reads can be
                # partition-shifted, SBUF-SBUF operands can't) and take
                # ln of the denominators straight from PSUM; both free the
                # psum bank quickly for the next pair.
                usb = usb_pool.tile([128, 1024], F16, tag="usb", name="usb")
                rb = rb_pool.tile([128, 1024], F32, tag="rb", name="rb")
                for hh in (0, 1):
                    with tc.high_priority(offset=300):
                        nc.vector.tensor_copy(
                            out=usb[64:128, hh * 512:(hh + 1) * 512],
                            in_=u[hh][0:64, :])
                    nc.scalar.activation(
                        out=rb[64:128, hh * 512:(hh + 1) * 512],
                        in_=u[hh][64:128, :],
                        func=mybir.ActivationFunctionType.Ln)
                # 1/l = exp(-ln(l)): ln and exp share one ACT table set.
                nc.scalar.activation(
                    out=rb[64:128, :], in_=rb[64:128, :],
                    func=mybir.ActivationFunctionType.Exp, scale=-1.0)
                ao = aoT_pool.tile([128, 512], F16, tag="aoT", name="aoT")
                for hh in (0, 1):
                    nc.vector.tensor_mul(
                        out=ao[hh * 64:(hh + 1) * 64, :],
                        in0=usb[64:128, hh * 512:(hh + 1) * 512],
                        in1=rb[64:128, hh * 512:(hh + 1) * 512])
                return ao

            def emit_oproj(qb, ao_pairs):
                for qt_l in range(4):
                    qt = 4 * qb + qt_l
                    op = spsum.tile([128, 1024], F32, tag="sp", name="op")
                    # hp outer / half inner: consecutive matmuls share the
                    # same stationary ao tile.
                    for hp in range(HH // 2):
                        for half in range(2):
                            nc.tensor.matmul(
                                op[:, half * 512:(half + 1) * 512],
                                lhsT=ao_pairs[hp][:, qt_l * 128:(qt_l + 1) * 128],
                                rhs=wo_sb[hp][:, half * 512:(half + 1) * 512],
                                start=(hp == 0), stop=(hp == 3),
                            )
                    osb = out_pool.tile([128, D], F16, tag="osb", name="osb")
                    nc.vector.tensor_copy(out=osb[:], in_=op[:])
                    nc.sync.dma_start(
                        out=out[qt * 128:(qt + 1) * 128, :], in_=osb[:])

            # Schedule: pairs run in DESCENDING q-block order.  Late blocks
            # are exp(ACT)-heavy relative to their matmul work, early blocks
            # the reverse, and the projection chains are pure PE -- so the
            # heaviest pairs start as early as possible (gated by only 3
            # chains) and all remaining chain/V work is spliced into them.
            # This keeps the ACT engine fed from ~10us on and keeps the PE
            # dense enough that the HAM clock-gate stays at full rate.
            assert n_qb == 4 and n_tbb == 2, "schedule hardcoded for S=2048"
            qk = emit_qk_chain
            emit_qk_chain(0, 0, True, pool=spsum)    # k(dq0, tbb0)
            emit_qk_chain(1, 0, False, pool=spsum)   # q(dq0, tbb1)
            emit_qk_chain(1, 0, True)                # k(dq0, tbb1)
            pairs = [(3, 0), (3, 1), (3, 2), (3, 3),
                     (2, 0), (2, 1), (2, 2), (2, 3),
                     (1, 0), (1, 1), (1, 2), (1, 3),
                     (0, 0), (0, 1), (0, 2), (0, 3)]
            # extra work spliced into each emit_pair call (call i covers
            # cur=pairs[i-1], nxt=pairs[i]; call 0 is the priming call).
            splices = {
                0: {t: [lambda t=t: emit_v_chain(t)] for t in range(4)},
                # nxt's scores (emitted from step 0) read kt tbb0 + qt tbb1,
                # so those chains MUST be spliced before step 0 (the tile
                # framework tracks write->read deps in emission order);
                # kt tbb1 is first read at step 8.
                1: {0: [lambda: qk(0, 1, True), lambda: qk(1, 1, False)],
                    2: [lambda: qk(1, 1, True)]},
                2: {0: [lambda: qk(0, 2, True), lambda: qk(1, 2, False)],
                    2: [lambda: qk(1, 2, True)]},
                3: {0: [lambda: qk(0, 3, True), lambda: qk(1, 3, False)],
                    2: [lambda: qk(1, 3, True)]},
                4: {0: [lambda: qk(0, 0, False)]},
                5: {0: [lambda: qk(0, 1, False)]},
                6: {0: [lambda: qk(0, 2, False)]},
                7: {0: [lambda: qk(0, 3, False)]},
            }
            for t in range(4, 16):
                splices[1].setdefault(t - 1, []).append(
                    lambda t=t: emit_v_chain(t))
            done_count = {qb: 0 for qb in range(n_qb)}
            pT_next = {}
            emit_pair(None, pairs[0], None, pT_next, extra=splices.get(0))
            ao_by_qb = {qb: [] for qb in range(n_qb)}
            oproj_queue = []
            for i, (qb, hp) in enumerate(pairs):
                pT_cur, pT_next = pT_next, {}
                nxt = pairs[i + 1] if i + 1 < len(pairs) else None
                if oproj_queue and oproj_queue[0][1] <= i:
                    oqb, _ = oproj_queue.pop(0)
                    emit_oproj(oqb, ao_by_qb.pop(oqb))
                ao_by_qb[qb].append(emit_pair(
                    (qb, hp), nxt, pT_cur, pT_next, extra=splices.get(i + 1)))
                done_count[qb] += 1
                if done_count[qb] == HH // 2:
                    oproj_queue.append((qb, i + 3))
            for oqb, _ in oproj_queue:
                emit_oproj(oqb, ao_by_qb.pop(oqb))

    nc.compile()
    return nc


_NC_CACHE = {}


def _get_nc(s=S):
    if s not in _NC_CACHE:
        _NC_CACHE[s] = build_nc(s)
    return _NC_CACHE[s]


def make_in_maps(x, w_q, w_k, w_v, w_o, s=S):
    """Host-side sharding: returns the 8 per-core input maps."""
    x = np.ascontiguousarray(np.asarray(x, dtype=np.float32))
    w_q = np.asarray(w_q, dtype=np.float32)
    w_k = np.asarray(w_k, dtype=np.float32)
    w_v = np.asarray(w_v, dtype=np.float32)
    w_o = np.asarray(w_o, dtype=np.float32)

    xTs = [np.ascontiguousarray(x[b].T.astype(np.float16)) for b in range(B)]
    wqTs = [np.ascontiguousarray(w_q[hg * HD:(hg + 1) * HD, :].T.astype(np.float16)) for hg in range(2)]
    wkTs = [np.ascontiguousarray(w_k[hg * HD:(hg + 1) * HD, :].T.astype(np.float16)) for hg in range(2)]
    wvTs = [np.ascontiguousarray(w_v[hg * HD:(hg + 1) * HD, :].T.astype(np.float16)) for hg in range(2)]
    woTs = [np.ascontiguousarray(w_o[:, hg * HD:(hg + 1) * HD].T.astype(np.float16)) for hg in range(2)]

    in_maps = []
    for c in range(N_CORES):
        b, hg = c // 2, c % 2
        in_maps.append({
            "xT": xTs[b], "wqT": wqTs[hg], "wkT": wkTs[hg],
            "wvT": wvTs[hg], "woT": woTs[hg],
        })
    return in_maps


def kernel(x, w_q, w_k, w_v, w_o, b_o):
    nc = _get_nc(S)
    in_maps = make_in_maps(x, w_q, w_k, w_v, w_o, s=S)
    res = run_bass_kernel_spmd(nc, in_maps, core_ids=list(range(N_CORES)))
    b_o = np.asarray(b_o, dtype=np.float32)
    outp = np.empty((B, S, D), dtype=np.float32)
    for b in range(B):
        outp[b] = (res.results[2 * b]["out"].astype(np.float32)
                   + res.results[2 * b + 1]["out"].astype(np.float32) + b_o)
    return outp


# revision 22
# speedup vs baseline: 1.0016x; 1.0016x over previous
"""Multi-head causal attention (B=4, S=2048, D=1024, H=16) on 8 TRN2 NeuronCores.

Sharding: core c handles batch b = c//2 and head-group hg = c%2 (8 heads each).
Each core computes Q/K/V projections for its (batch, head-group), causal
attention, and a partial output projection over its 512 head-dims.  The host
sums the two partials per batch and adds b_o.  No collectives.

Device-side layout choices:
  - x is passed transposed (xT [D, S]), loaded once into SBUF and reused by
    all projection chains.
  - Q and K are produced transposed (QT/KT [dq, S]); scores are computed
    transposed (S^T [kpos, q]).  K^T tiles keep the natural head-pair packing
    (head 2d in rows 0-63, head 2d+1 in rows 64-127) and the two heads'
    score matmuls run CONCURRENTLY on the PE via row tiling (64-row
    contraction each, tile_position (0,0) / (64,0)).
  - Scores for one kt tile and both heads land in one 2-bank [128, 1024]
    PSUM tile, so a single ACT exp covers both heads (halves ACT instruction
    overhead, the co-bottleneck).
  - No max-subtraction in softmax: scaled scores are ~N(0,1), exp is safe.
  - AV matmuls trim the below-diagonal (always-zero) query range instead of
    memsetting P; only the 128-wide diagonal square needs affine_select.
  - U (64 value dims + 64 denominator-broadcast rows, via ones columns in V)
    is copied out of PSUM to SBUF in fp16 right away so only 2 PSUM banks of
    accumulators are needed; 1/l = exp(-ln(l)) runs on the SBUF copy off the
    critical path (ln and exp share one ACT table set -> no table reloads).
"""

import sys
import os

sys.path.insert(0, "/opt/trn_rl_repo")

import numpy as np

import concourse.bacc as bacc
import concourse.mybir as mybir
import concourse.tile as tile
from concourse.bass_utils import run_bass_kernel_spmd

# The ACT table-load pass resolves each activation to the first table set
# containing it, which puts Exp (exp_and_others) and Ln
# (natural_log_exp_and_others) in different sets and reloads tables at every
# softmax normalization.  Restrict Exp/Ln to the one set that holds both so
# the whole kernel runs off a single table load.
_orig_get_tables = bacc.get_activation_tables


def _patched_tables(arch):
    t = _orig_get_tables(arch)
    for name, fns in t.items():
        if name != "natural_log_exp_and_others":
            fns.discard(mybir.ActivationFunctionType.Exp)
            fns.discard(mybir.ActivationFunctionType.Ln)
    return t


bacc.get_activation_tables = _patched_tables

B, S, D, H = 4, 2048, 1024, 16
DK = D // H          # 64
HH = H // 2          # 8 heads per core
HD = HH * DK         # 512 head-dims per core
N_CORES = 8

F32 = mybir.dt.float32
F16 = mybir.dt.float16

SCALE = 1.0 / np.sqrt(DK)


def build_nc(s=S):
    """Build the per-core SPMD program.  `s` is the sequence length (tunable
    for small-scale simulation; must be a multiple of 1024)."""
    assert s % 1024 == 0
    n_qb = s // 512          # 512-wide q blocks
    n_t128 = s // 128        # 128-wide token tiles
    n_tbb = s // 1024        # 1024-wide token blocks (projection chains)
    n_dt = D // 128          # din tiles (8)

    nc = bacc.Bacc("TRN2", target_bir_lowering=False, debug=False,
                   num_devices=N_CORES)

    xT = nc.dram_tensor("xT", [D, s], F16, kind="ExternalInput")
    wqT = nc.dram_tensor("wqT", [D, HD], F16, kind="ExternalInput")
    wkT = nc.dram_tensor("wkT", [D, HD], F16, kind="ExternalInput")
    wvT = nc.dram_tensor("wvT", [D, HD], F16, kind="ExternalInput")
    woT = nc.dram_tensor("woT", [HD, D], F16, kind="ExternalInput")
    out = nc.dram_tensor("out", [s, D], F16, kind="ExternalOutput")

    with tile.TileContext(nc) as tc:
        with tc.tile_pool(name="persist", bufs=1) as persist, \
             tc.tile_pool(name="pT", bufs=20) as pT_pool, \
             tc.tile_pool(name="usb", bufs=3) as usb_pool, \
             tc.tile_pool(name="rb", bufs=2) as rb_pool, \
             tc.tile_pool(name="aoT", bufs=12) as aoT_pool, \
             tc.tile_pool(name="outsb", bufs=2) as out_pool, \
             tc.tile_pool(name="bigp", bufs=3, space="PSUM") as bigp, \
             tc.tile_pool(name="upsum", bufs=2, space="PSUM") as upsum:

            # Persistent SBUF arrays (live for the whole kernel).
            # Q^T / K^T per head-pair: head 2d in rows 0-63, 2d+1 in 64-127.
            qt_sb = [persist.tile([128, s], F16, tag=f"qt{d}", name=f"qt{d}") for d in range(HD // 128)]
            kt_sb = [persist.tile([128, s], F16, tag=f"kt{d}", name=f"kt{d}") for d in range(HD // 128)]
            # V tiles hold [t, head, 2*dk]: cols 0-63 are V, cols 64-127 are
            # 1.0.  As the AV stationary this makes the matmul emit U^T on
            # psum rows 0-63 and the softmax denominator on rows 64-127.
            v_sb = [persist.tile([128, HH, 2 * DK], F16, tag=f"v{t}", name=f"v{t}") for t in range(n_t128)]
            wo_sb = [persist.tile([128, D], F16, tag=f"wo{d}", name=f"wo{d}") for d in range(HD // 128)]
            wv_sb = [persist.tile([128, HD], F16, tag=f"wv{i}", name=f"wv{i}") for i in range(n_dt)]
            # x resident: [tbb][i] -> [128, 1024] (din tile i, token block tbb)
            x_sb = [[persist.tile([128, 1024], F16, tag=f"x{tbb}_{i}", name=f"x{tbb}_{i}")
                     for i in range(n_dt)] for tbb in range(n_tbb)]
            # Q/K weights in [128, 256] column slices (dq pairs) so the first
            # chain is gated on only 8 small DMAs.
            w_tiles = {}
            for wkey in ("q", "k"):
                for i in range(n_dt):
                    for dqp in range(HD // 256):
                        w_tiles[(wkey, i, dqp)] = persist.tile(
                            [128, 256], F16, tag=f"w{wkey}{i}_{dqp}",
                            name=f"w{wkey}{i}_{dqp}")

            # Ones columns of the V tiles (written once; V chains only write
            # cols 0-63).  On gpsimd to keep the DVE free.
            for t in range(n_t128):
                nc.gpsimd.memset(v_sb[t][:, :, DK:2 * DK], 1.0)

            # Warmup matmuls on scratch data: keep the PE busy through the
            # initial DMA lead-in so the HAM clock-gate is warm (2.4 GHz)
            # when real work starts.  Scratch memset on the (idle) DVE so
            # the warmup isn't queued behind the gpsimd memsets.
            scratch = persist.tile([128, 512], F16, tag="scratch", name="scratch")
            nc.vector.memset(scratch[:], 0.0)
            wm = bigp.tile([128, 1024], F32, tag="big", name="wm")
            for _ in range(40):
                nc.tensor.matmul(wm[:, 0:512], lhsT=scratch[:, 0:128],
                                 rhs=scratch[:], start=True, stop=True)

            # DMAs in first-use order for the descending-block schedule:
            # k(dq0,tbb0) gates the first chain, then q(dq0,tbb1) needs wq
            # dqp0 + x tbb1, then wv (V chains start during the priming
            # pair), then the dq2-3 weights, then wo (out-proj is late).
            for i in range(n_dt):
                nc.sync.dma_start(
                    out=w_tiles[("k", i, 0)][:],
                    in_=wkT[i * 128:(i + 1) * 128, 0:256])
                nc.sync.dma_start(
                    out=x_sb[0][i][:], in_=xT[i * 128:(i + 1) * 128, 0:1024])
            for i in range(n_dt):
                nc.sync.dma_start(
                    out=w_tiles[("q", i, 0)][:],
                    in_=wqT[i * 128:(i + 1) * 128, 0:256])
                if n_tbb > 1:
                    nc.sync.dma_start(
                        out=x_sb[1][i][:],
                        in_=xT[i * 128:(i + 1) * 128, 1024:2048])
            for i in range(n_dt):
                nc.sync.dma_start(out=wv_sb[i][:], in_=wvT[i * 128:(i + 1) * 128, :])
            for dqp in range(1, HD // 256):
                for wkey, wdram in (("k", wkT), ("q", wqT)):
                    for i in range(n_dt):
                        nc.sync.dma_start(
                            out=w_tiles[(wkey, i, dqp)][:],
                            in_=wdram[i * 128:(i + 1) * 128,
                                      dqp * 256:(dqp + 1) * 256])
            for d in range(HD // 128):
                nc.sync.dma_start(out=wo_sb[d][:], in_=woT[d * 128:(d + 1) * 128, :])

            def emit_qk_chain(tbb, dq, is_k):
                """Q^T or K^T projection for one 1024-token block and one
                head-pair dq, off the resident x tiles."""
                wkey = "k" if is_k else "q"
                ps = bigp.tile([128, 1024], F32, tag="big", name="qk")
                # i outer / half inner: consecutive matmuls share the same
                # stationary weights.
                for i in range(n_dt):
                    w = w_tiles[(wkey, i, dq // 2)][:, (dq % 2) * 128:(dq % 2 + 1) * 128]
                    for half in range(2):
                        nc.tensor.matmul(
                            ps[:, half * 512:(half + 1) * 512],
                            lhsT=w,
                            rhs=x_sb[tbb][i][:, half * 512:(half + 1) * 512],
                            start=(i == 0), stop=(i == n_dt - 1),
                        )
                dst = kt_sb[dq] if is_k else qt_sb[dq]
                nc.vector.tensor_copy(
                    out=dst[:, tbb * 1024:(tbb + 1) * 1024], in_=ps[:])

            def emit_v_chain(tb):
                """V projection for one 128-token tile, spliced into the
                attention stream just before the q-block that needs it."""
                vp = bigp.tile([128, 1024], F32, tag="big", name="vp")
                for i in range(n_dt):
                    xs = x_sb[tb // 8][i][:, (tb % 8) * 128:(tb % 8 + 1) * 128]
                    nc.tensor.matmul(
                        vp[:, 0:512], lhsT=xs, rhs=wv_sb[i][:],
                        start=(i == 0), stop=(i == n_dt - 1),
                    )
                nc.vector.tensor_copy(
                    out=v_sb[tb][:, :, 0:DK],
                    in_=vp[:, 0:512].rearrange("p (h k) -> p h k", h=HH))

            def emit_score_kt(qb, hp, kt, pT):
                """Scores + exp for one kt tile, BOTH heads of the pair:
                row-tiled matmuls (64-contraction each) into one 2-bank psum
                tile, one exp over both."""
                lo = max(kt - 4 * qb, 0) * 128
                sp = bigp.tile([128, 1024], F32, tag="big", name="sp")
                for hh in (0, 1):
                    nc.tensor.matmul(
                        sp[:, hh * 512 + lo:(hh + 1) * 512],
                        lhsT=kt_sb[hp][hh * 64:(hh + 1) * 64,
                                       kt * 128:(kt + 1) * 128],
                        rhs=qt_sb[hp][hh * 64:(hh + 1) * 64,
                                      qb * 512 + lo:(qb + 1) * 512],
                        start=True, stop=True,
                    )
                p = pT_pool.tile([128, 1024], F16, tag="p", name="p")
                nc.scalar.activation(
                    out=p[:, lo:1024], in_=sp[:, lo:1024],
                    func=mybir.ActivationFunctionType.Exp,
                    scale=float(SCALE))
                if kt >= 4 * qb:
                    # zero strict-upper (kpos > q) region of the 128-wide
                    # diagonal square, per head
                    for hh in (0, 1):
                        nc.gpsimd.affine_select(
                            out=p[:, hh * 512 + lo:hh * 512 + lo + 128],
                            in_=p[:, hh * 512 + lo:hh * 512 + lo + 128],
                            compare_op=mybir.AluOpType.is_ge,
                            fill=0.0, base=0, channel_multiplier=-1,
                            pattern=[[1, 128]])
                pT[kt] = (p, lo)

            def emit_pair(cur, nxt, pT_cur, pT_next, extra=None):
                """Interleave next pair's scores with current pair's AV
                chains at kt granularity: the PE gets AV matmuls to run
                while the ACT engine works through the scores' exps.
                `extra` maps a kt step to closures (projection/V chains)
                spliced in at that step."""
                nkt_cur = 4 * cur[0] + 4 if cur else 0
                nkt_nxt = 4 * nxt[0] + 4 if nxt else 0
                u = {}
                if cur:
                    for hh in (0, 1):
                        u[hh] = upsum.tile([128, 512], F32, tag="u", name="u")
                nsteps = max(nkt_cur, nkt_nxt)
                for kt in range(nsteps):
                    for work in (extra or {}).get(kt, ()):
                        work()
                    if kt < nkt_nxt:
                        emit_score_kt(nxt[0], nxt[1], kt, pT_next)
                    if kt < nkt_cur:
                        p, lo = pT_cur[kt]
                        for hh in (0, 1):
                            nc.tensor.matmul(
                                u[hh][:, lo:512],
                                lhsT=v_sb[kt][:, 2 * cur[1] + hh, :],
                                rhs=p[:, hh * 512 + lo:(hh + 1) * 512],
                                start=(kt == 0), stop=(kt == nkt_cur - 1),
                            )
                for work in (extra or {}).get(nsteps, ()):
                    work()
                if not cur:
                    return None
                # Evacuate U to SBUF fp16 (rows 64-127: PSUM reads can be
                # partition-shifted, SBUF-SBUF operands can't) and take
                # ln of the denominators straight from PSUM; both free the
                # psum bank quickly for the next pair.
                usb = usb_pool.tile([128, 1024], F16, tag="usb", name="usb")
                rb = rb_pool.tile([128, 1024], F32, tag="rb", name="rb")
                for hh in (0, 1):
                    with tc.high_priority(offset=300):
                        nc.vector.tensor_copy(
                            out=usb[64:128, hh * 512:(hh + 1) * 512],
                            in_=u[hh][0:64, :])
                    nc.scalar.activation(
                        out=rb[64:128, hh * 512:(hh + 1) * 512],
                        in_=u[hh][64:128, :],
                        func=mybir.ActivationFunctionType.Ln)
                # 1/l = exp(-ln(l)): ln and exp share one ACT table set.
                nc.scalar.activation(
                    out=rb[64:128, :], in_=rb[64:128, :],
                    func=mybir.ActivationFunctionType.Exp, scale=-1.0)
                ao = aoT_pool.tile([128, 512], F16, tag="aoT", name="aoT")
                for hh in (0, 1):
                    nc.vector.tensor_mul(
                        out=ao[hh * 64:(hh + 1) * 64, :],
                        in0=usb[64:128, hh * 512:(hh + 1) * 512],
                        in1=rb[64:128, hh * 512:(hh + 1) * 512])
                return ao

            def emit_oproj(qb, ao_pairs):
                for qt_l in range(4):
                    qt = 4 * qb + qt_l
                    op = bigp.tile([128, 1024], F32, tag="big", name="op")
                    # hp outer / half inner: consecutive matmuls share the
                    # same stationary ao tile.
                    for hp in range(HH // 2):
                        for half in range(2):
                            nc.tensor.matmul(
                                op[:, half * 512:(half + 1) * 512],
                                lhsT=ao_pairs[hp][:, qt_l * 128:(qt_l + 1) * 128],
                                rhs=wo_sb[hp][:, half * 512:(half + 1) * 512],
                                start=(hp == 0), stop=(hp == 3),
                            )
                    osb = out_pool.tile([128, D], F16, tag="osb", name="osb")
                    nc.vector.tensor_copy(out=osb[:], in_=op[:])
                    nc.sync.dma_start(
                        out=out[qt * 128:(qt + 1) * 128, :], in_=osb[:])

            # Schedule: pairs run in DESCENDING q-block order.  Late blocks
            # are exp(ACT)-heavy relative to their matmul work, early blocks
            # the reverse, and the projection chains are pure PE -- so the
            # heaviest pairs start as early as possible (gated by only 3
            # chains) and all remaining chain/V work is spliced into them.
            # This keeps the ACT engine fed from ~10us on and keeps the PE
            # dense enough that the HAM clock-gate stays at full rate.
            assert n_qb == 4 and n_tbb == 2, "schedule hardcoded for S=2048"
            qk = emit_qk_chain
            emit_qk_chain(0, 0, True)    # k(dq0, tbb0)
            emit_qk_chain(1, 0, False)   # q(dq0, tbb1)
            emit_qk_chain(1, 0, True)    # k(dq0, tbb1)
            pairs = [(3, 0), (3, 1), (1, 0), (3, 2),
                     (1, 1), (3, 3), (2, 0), (2, 1),
                     (2, 2), (2, 3), (1, 2), (1, 3),
                     (0, 0), (0, 1), (0, 2), (0, 3)]
            # extra work spliced into each emit_pair call (call i covers
            # cur=pairs[i-1], nxt=pairs[i]; call 0 is the priming call).
            # nxt's scores (emitted from step 0) read kt tbb0 + qt, so
            # those chains MUST be spliced before step 0 (the tile
            # framework tracks write->read deps in emission order);
            # kt tbb1 is first read at step 8.
            splices = {
                0: {t: [lambda t=t: emit_v_chain(t)] for t in range(4)},
                1: {0: [lambda: qk(0, 1, True), lambda: qk(1, 1, False)],
                    2: [lambda: qk(1, 1, True)],
                    4: [lambda: qk(0, 0, False)]},
                2: {0: [lambda: qk(0, 2, True), lambda: qk(1, 2, False)],
                    2: [lambda: qk(1, 2, True)]},
                3: {0: [lambda: qk(0, 1, False)]},
                4: {0: [lambda: qk(0, 3, True), lambda: qk(1, 3, False)],
                    2: [lambda: qk(1, 3, True)]},
                9: {0: [lambda: qk(0, 2, False)]},
                10: {0: [lambda: qk(0, 3, False)]},
            }
            for t in range(4, 16):
                splices[1].setdefault(t - 1, []).append(
                    lambda t=t: emit_v_chain(t))
            done_count = {qb: 0 for qb in range(n_qb)}
            pT_next = {}
            emit_pair(None, pairs[0], None, pT_next, extra=splices.get(0))
            ao_by_qb = {qb: [] for qb in range(n_qb)}
            oproj_queue = []
            for i, (qb, hp) in enumerate(pairs):
                pT_cur, pT_next = pT_next, {}
                nxt = pairs[i + 1] if i + 1 < len(pairs) else None
                if oproj_queue and oproj_queue[0][1] <= i:
                    oqb, _ = oproj_queue.pop(0)
                    emit_oproj(oqb, ao_by_qb.pop(oqb))
                ao_by_qb[qb].append(emit_pair(
                    (qb, hp), nxt, pT_cur, pT_next, extra=splices.get(i + 1)))
                done_count[qb] += 1
                if done_count[qb] == HH // 2:
                    oproj_queue.append((qb, i + 3))
            for oqb, _ in oproj_queue:
                emit_oproj(oqb, ao_by_qb.pop(oqb))

    nc.compile()
    return nc


_NC_CACHE = {}


def _get_nc(s=S):
    if s not in _NC_CACHE:
        _NC_CACHE[s] = build_nc(s)
    return _NC_CACHE[s]


def make_in_maps(x, w_q, w_k, w_v, w_o, s=S):
    """Host-side sharding: returns the 8 per-core input maps."""
    x = np.ascontiguousarray(np.asarray(x, dtype=np.float32))
    w_q = np.asarray(w_q, dtype=np.float32)
    w_k = np.asarray(w_k, dtype=np.float32)
    w_v = np.asarray(w_v, dtype=np.float32)
    w_o = np.asarray(w_o, dtype=np.float32)

    xTs = [np.ascontiguousarray(x[b].T.astype(np.float16)) for b in range(B)]
    wqTs = [np.ascontiguousarray(w_q[hg * HD:(hg + 1) * HD, :].T.astype(np.float16)) for hg in range(2)]
    wkTs = [np.ascontiguousarray(w_k[hg * HD:(hg + 1) * HD, :].T.astype(np.float16)) for hg in range(2)]
    wvTs = [np.ascontiguousarray(w_v[hg * HD:(hg + 1) * HD, :].T.astype(np.float16)) for hg in range(2)]
    woTs = [np.ascontiguousarray(w_o[:, hg * HD:(hg + 1) * HD].T.astype(np.float16)) for hg in range(2)]

    in_maps = []
    for c in range(N_CORES):
        b, hg = c // 2, c % 2
        in_maps.append({
            "xT": xTs[b], "wqT": wqTs[hg], "wkT": wkTs[hg],
            "wvT": wvTs[hg], "woT": woTs[hg],
        })
    return in_maps


def kernel(x, w_q, w_k, w_v, w_o, b_o):
    nc = _get_nc(S)
    in_maps = make_in_maps(x, w_q, w_k, w_v, w_o, s=S)
    res = run_bass_kernel_spmd(nc, in_maps, core_ids=list(range(N_CORES)))
    b_o = np.asarray(b_o, dtype=np.float32)
    outp = np.empty((B, S, D), dtype=np.float32)
    for b in range(B):
        outp[b] = (res.results[2 * b]["out"].astype(np.float32)
                   + res.results[2 * b + 1]["out"].astype(np.float32) + b_o)
    return outp


# revision 23
# speedup vs baseline: 1.0139x; 1.0123x over previous
"""Multi-head causal attention (B=4, S=2048, D=1024, H=16) on 8 TRN2 NeuronCores.

Sharding: core c handles batch b = c//2 and head-group hg = c%2 (8 heads each).
Each core computes Q/K/V projections for its (batch, head-group), causal
attention, and a partial output projection over its 512 head-dims.  The host
sums the two partials per batch and adds b_o.  No collectives.

Device-side layout choices:
  - x is passed transposed (xT [D, S]), loaded once into SBUF and reused by
    all projection chains.
  - Q and K are produced transposed (QT/KT [dq, S]); scores are computed
    transposed (S^T [kpos, q]).  K^T tiles keep the natural head-pair packing
    (head 2d in rows 0-63, head 2d+1 in rows 64-127) and the two heads'
    score matmuls run CONCURRENTLY on the PE via row tiling (64-row
    contraction each, tile_position (0,0) / (64,0)).
  - Scores for one kt tile and both heads land in one 2-bank [128, 1024]
    PSUM tile, so a single ACT exp covers both heads (halves ACT instruction
    overhead, the co-bottleneck).
  - No max-subtraction in softmax: scaled scores are ~N(0,1), exp is safe.
  - AV matmuls trim the below-diagonal (always-zero) query range instead of
    memsetting P; only the 128-wide diagonal square needs affine_select.
  - U (64 value dims + 64 denominator-broadcast rows, via ones columns in V)
    is copied out of PSUM to SBUF in fp16 right away so only 2 PSUM banks of
    accumulators are needed; 1/l = exp(-ln(l)) runs on the SBUF copy off the
    critical path (ln and exp share one ACT table set -> no table reloads).
"""

import sys
import os

sys.path.insert(0, "/opt/trn_rl_repo")

import numpy as np

import concourse.bacc as bacc
import concourse.mybir as mybir
import concourse.tile as tile
from concourse.bass_utils import run_bass_kernel_spmd

# The ACT table-load pass resolves each activation to the first table set
# containing it, which puts Exp (exp_and_others) and Ln
# (natural_log_exp_and_others) in different sets and reloads tables at every
# softmax normalization.  Restrict Exp/Ln to the one set that holds both so
# the whole kernel runs off a single table load.
_orig_get_tables = bacc.get_activation_tables


def _patched_tables(arch):
    t = _orig_get_tables(arch)
    for name, fns in t.items():
        if name != "natural_log_exp_and_others":
            fns.discard(mybir.ActivationFunctionType.Exp)
            fns.discard(mybir.ActivationFunctionType.Ln)
    return t


bacc.get_activation_tables = _patched_tables

B, S, D, H = 4, 2048, 1024, 16
DK = D // H          # 64
HH = H // 2          # 8 heads per core
HD = HH * DK         # 512 head-dims per core
N_CORES = 8

F32 = mybir.dt.float32
F16 = mybir.dt.float16

SCALE = 1.0 / np.sqrt(DK)


def build_nc(s=S):
    """Build the per-core SPMD program.  `s` is the sequence length (tunable
    for small-scale simulation; must be a multiple of 1024)."""
    assert s % 1024 == 0
    n_qb = s // 512          # 512-wide q blocks
    n_t128 = s // 128        # 128-wide token tiles
    n_tbb = s // 1024        # 1024-wide token blocks (projection chains)
    n_dt = D // 128          # din tiles (8)

    nc = bacc.Bacc("TRN2", target_bir_lowering=False, debug=False,
                   num_devices=N_CORES)

    xT = nc.dram_tensor("xT", [D, s], F16, kind="ExternalInput")
    wqT = nc.dram_tensor("wqT", [D, HD], F16, kind="ExternalInput")
    wkT = nc.dram_tensor("wkT", [D, HD], F16, kind="ExternalInput")
    wvT = nc.dram_tensor("wvT", [D, HD], F16, kind="ExternalInput")
    woT = nc.dram_tensor("woT", [HD, D], F16, kind="ExternalInput")
    out = nc.dram_tensor("out", [s, D], F16, kind="ExternalOutput")

    with tile.TileContext(nc) as tc:
        with tc.tile_pool(name="persist", bufs=1) as persist, \
             tc.tile_pool(name="pT", bufs=20) as pT_pool, \
             tc.tile_pool(name="usb", bufs=3) as usb_pool, \
             tc.tile_pool(name="rb", bufs=2) as rb_pool, \
             tc.tile_pool(name="aoT", bufs=12) as aoT_pool, \
             tc.tile_pool(name="outsb", bufs=2) as out_pool, \
             tc.tile_pool(name="bigp", bufs=3, space="PSUM") as bigp, \
             tc.tile_pool(name="upsum", bufs=2, space="PSUM") as upsum:

            # Persistent SBUF arrays (live for the whole kernel).
            # Q^T / K^T per head-pair: head 2d in rows 0-63, 2d+1 in 64-127.
            qt_sb = [persist.tile([128, s], F16, tag=f"qt{d}", name=f"qt{d}") for d in range(HD // 128)]
            kt_sb = [persist.tile([128, s], F16, tag=f"kt{d}", name=f"kt{d}") for d in range(HD // 128)]
            # V tiles hold [t, head, 2*dk]: cols 0-63 are V, cols 64-127 are
            # 1.0.  As the AV stationary this makes the matmul emit U^T on
            # psum rows 0-63 and the softmax denominator on rows 64-127.
            v_sb = [persist.tile([128, HH, 2 * DK], F16, tag=f"v{t}", name=f"v{t}") for t in range(n_t128)]
            wo_sb = [persist.tile([128, D], F16, tag=f"wo{d}", name=f"wo{d}") for d in range(HD // 128)]
            wv_sb = [persist.tile([128, HD], F16, tag=f"wv{i}", name=f"wv{i}") for i in range(n_dt)]
            # x resident: [tbb][i] -> [128, 1024] (din tile i, token block tbb)
            x_sb = [[persist.tile([128, 1024], F16, tag=f"x{tbb}_{i}", name=f"x{tbb}_{i}")
                     for i in range(n_dt)] for tbb in range(n_tbb)]
            # Q/K weights in [128, 256] column slices (dq pairs) so the first
            # chain is gated on only 8 small DMAs.
            w_tiles = {}
            for wkey in ("q", "k"):
                for i in range(n_dt):
                    for dqp in range(HD // 256):
                        w_tiles[(wkey, i, dqp)] = persist.tile(
                            [128, 256], F16, tag=f"w{wkey}{i}_{dqp}",
                            name=f"w{wkey}{i}_{dqp}")

            # Ones columns of the V tiles (written once; V chains only write
            # cols 0-63).  On gpsimd to keep the DVE free.
            for t in range(n_t128):
                nc.gpsimd.memset(v_sb[t][:, :, DK:2 * DK], 1.0)

            # Warmup matmuls on scratch data: keep the PE busy through the
            # initial DMA lead-in so the HAM clock-gate is warm (2.4 GHz)
            # when real work starts.  Scratch memset on the (idle) DVE so
            # the warmup isn't queued behind the gpsimd memsets.
            scratch = persist.tile([128, 512], F16, tag="scratch", name="scratch")
            nc.vector.memset(scratch[:], 0.0)
            wm = bigp.tile([128, 1024], F32, tag="big", name="wm")
            for _ in range(40):
                nc.tensor.matmul(wm[:, 0:512], lhsT=scratch[:, 0:128],
                                 rhs=scratch[:], start=True, stop=True)

            # DMAs in first-use order for the descending-block schedule:
            # k(dq0,tbb0) gates the first chain, then q(dq0,tbb1) needs wq
            # dqp0 + x tbb1, then wv (V chains start during the priming
            # pair), then the dq2-3 weights, then wo (out-proj is late).
            for i in range(n_dt):
                nc.sync.dma_start(
                    out=w_tiles[("k", i, 0)][:],
                    in_=wkT[i * 128:(i + 1) * 128, 0:256])
                nc.sync.dma_start(
                    out=x_sb[0][i][:], in_=xT[i * 128:(i + 1) * 128, 0:1024])
            for i in range(n_dt):
                nc.sync.dma_start(
                    out=w_tiles[("q", i, 0)][:],
                    in_=wqT[i * 128:(i + 1) * 128, 0:256])
                if n_tbb > 1:
                    nc.sync.dma_start(
                        out=x_sb[1][i][:],
                        in_=xT[i * 128:(i + 1) * 128, 1024:2048])
            for i in range(n_dt):
                nc.sync.dma_start(out=wv_sb[i][:], in_=wvT[i * 128:(i + 1) * 128, :])
            for dqp in range(1, HD // 256):
                for wkey, wdram in (("k", wkT), ("q", wqT)):
                    for i in range(n_dt):
                        nc.sync.dma_start(
                            out=w_tiles[(wkey, i, dqp)][:],
                            in_=wdram[i * 128:(i + 1) * 128,
                                      dqp * 256:(dqp + 1) * 256])
            for d in range(HD // 128):
                nc.sync.dma_start(out=wo_sb[d][:], in_=woT[d * 128:(d + 1) * 128, :])

            def emit_qk_chain(tbb, dq, is_k):
                """Q^T or K^T projection for one 1024-token block and one
                head-pair dq, off the resident x tiles."""
                wkey = "k" if is_k else "q"
                ps = bigp.tile([128, 1024], F32, tag="big", name="qk")
                # i outer / half inner: consecutive matmuls share the same
                # stationary weights.
                for i in range(n_dt):
                    w = w_tiles[(wkey, i, dq // 2)][:, (dq % 2) * 128:(dq % 2 + 1) * 128]
                    for half in range(2):
                        nc.tensor.matmul(
                            ps[:, half * 512:(half + 1) * 512],
                            lhsT=w,
                            rhs=x_sb[tbb][i][:, half * 512:(half + 1) * 512],
                            start=(i == 0), stop=(i == n_dt - 1),
                        )
                dst = kt_sb[dq] if is_k else qt_sb[dq]
                nc.vector.tensor_copy(
                    out=dst[:, tbb * 1024:(tbb + 1) * 1024], in_=ps[:])

            def emit_v_chain(tb):
                """V projection for one 128-token tile, spliced into the
                attention stream just before the q-block that needs it."""
                vp = bigp.tile([128, 1024], F32, tag="big", name="vp")
                for i in range(n_dt):
                    xs = x_sb[tb // 8][i][:, (tb % 8) * 128:(tb % 8 + 1) * 128]
                    nc.tensor.matmul(
                        vp[:, 0:512], lhsT=xs, rhs=wv_sb[i][:],
                        start=(i == 0), stop=(i == n_dt - 1),
                    )
                nc.vector.tensor_copy(
                    out=v_sb[tb][:, :, 0:DK],
                    in_=vp[:, 0:512].rearrange("p (h k) -> p h k", h=HH))

            def emit_score_kt(qb, hp, kt, pT):
                """Scores + exp for one kt tile, BOTH heads of the pair:
                row-tiled matmuls (64-contraction each) into one 2-bank psum
                tile, one exp over both."""
                lo = max(kt - 4 * qb, 0) * 128
                sp = bigp.tile([128, 1024], F32, tag="big", name="sp")
                for hh in (0, 1):
                    nc.tensor.matmul(
                        sp[:, hh * 512 + lo:(hh + 1) * 512],
                        lhsT=kt_sb[hp][hh * 64:(hh + 1) * 64,
                                       kt * 128:(kt + 1) * 128],
                        rhs=qt_sb[hp][hh * 64:(hh + 1) * 64,
                                      qb * 512 + lo:(qb + 1) * 512],
                        start=True, stop=True,
                    )
                p = pT_pool.tile([128, 1024], F16, tag="p", name="p")
                nc.scalar.activation(
                    out=p[:, lo:1024], in_=sp[:, lo:1024],
                    func=mybir.ActivationFunctionType.Exp,
                    scale=float(SCALE))
                if kt >= 4 * qb:
                    # zero strict-upper (kpos > q) region of the 128-wide
                    # diagonal square, per head
                    for hh in (0, 1):
                        nc.gpsimd.affine_select(
                            out=p[:, hh * 512 + lo:hh * 512 + lo + 128],
                            in_=p[:, hh * 512 + lo:hh * 512 + lo + 128],
                            compare_op=mybir.AluOpType.is_ge,
                            fill=0.0, base=0, channel_multiplier=-1,
                            pattern=[[1, 128]])
                pT[kt] = (p, lo)

            def emit_pair(cur, nxt, pT_cur, pT_next, extra=None):
                """Interleave next pair's scores with current pair's AV
                chains at kt granularity: the PE gets AV matmuls to run
                while the ACT engine works through the scores' exps.
                `extra` maps a kt step to closures (projection/V chains)
                spliced in at that step."""
                nkt_cur = 4 * cur[0] + 4 if cur else 0
                nkt_nxt = 4 * nxt[0] + 4 if nxt else 0
                u = {}
                if cur:
                    for hh in (0, 1):
                        u[hh] = upsum.tile([128, 512], F32, tag="u", name="u")
                nsteps = max(nkt_cur, nkt_nxt)
                for kt in range(nsteps):
                    for work in (extra or {}).get(kt, ()):
                        work()
                    if kt < nkt_nxt:
                        emit_score_kt(nxt[0], nxt[1], kt, pT_next)
                    if kt < nkt_cur:
                        p, lo = pT_cur[kt]
                        for hh in (0, 1):
                            nc.tensor.matmul(
                                u[hh][:, lo:512],
                                lhsT=v_sb[kt][:, 2 * cur[1] + hh, :],
                                rhs=p[:, hh * 512 + lo:(hh + 1) * 512],
                                start=(kt == 0), stop=(kt == nkt_cur - 1),
                            )
                for work in (extra or {}).get(nsteps, ()):
                    work()
                if not cur:
                    return None
                # Evacuate U to SBUF fp16 (rows 64-127: PSUM reads can be
                # partition-shifted, SBUF-SBUF operands can't) and take
                # ln of the denominators straight from PSUM; both free the
                # psum bank quickly for the next pair.
                usb = usb_pool.tile([128, 1024], F16, tag="usb", name="usb")
                rb = rb_pool.tile([128, 1024], F32, tag="rb", name="rb")
                for hh in (0, 1):
                    with tc.high_priority(offset=300):
                        nc.vector.tensor_copy(
                            out=usb[64:128, hh * 512:(hh + 1) * 512],
                            in_=u[hh][0:64, :])
                    nc.scalar.activation(
                        out=rb[64:128, hh * 512:(hh + 1) * 512],
                        in_=u[hh][64:128, :],
                        func=mybir.ActivationFunctionType.Ln)
                # 1/l = exp(-ln(l)): ln and exp share one ACT table set.
                nc.scalar.activation(
                    out=rb[64:128, :], in_=rb[64:128, :],
                    func=mybir.ActivationFunctionType.Exp, scale=-1.0)
                ao = aoT_pool.tile([128, 512], F16, tag="aoT", name="aoT")
                for hh in (0, 1):
                    nc.vector.tensor_mul(
                        out=ao[hh * 64:(hh + 1) * 64, :],
                        in0=usb[64:128, hh * 512:(hh + 1) * 512],
                        in1=rb[64:128, hh * 512:(hh + 1) * 512])
                return ao

            def emit_oproj(qb, ao_pairs):
                for qt_l in range(4):
                    qt = 4 * qb + qt_l
                    op = bigp.tile([128, 1024], F32, tag="big", name="op")
                    # hp outer / half inner: consecutive matmuls share the
                    # same stationary ao tile.
                    for hp in range(HH // 2):
                        for half in range(2):
                            nc.tensor.matmul(
                                op[:, half * 512:(half + 1) * 512],
                                lhsT=ao_pairs[hp][:, qt_l * 128:(qt_l + 1) * 128],
                                rhs=wo_sb[hp][:, half * 512:(half + 1) * 512],
                                start=(hp == 0), stop=(hp == 3),
                            )
                    osb = out_pool.tile([128, D], F16, tag="osb", name="osb")
                    nc.vector.tensor_copy(out=osb[:], in_=op[:])
                    nc.sync.dma_start(
                        out=out[qt * 128:(qt + 1) * 128, :], in_=osb[:])

            # Schedule: pairs run in DESCENDING q-block order.  Late blocks
            # are exp(ACT)-heavy relative to their matmul work, early blocks
            # the reverse, and the projection chains are pure PE -- so the
            # heaviest pairs start as early as possible (gated by only 3
            # chains) and all remaining chain/V work is spliced into them.
            # This keeps the ACT engine fed from ~10us on and keeps the PE
            # dense enough that the HAM clock-gate stays at full rate.
            assert n_qb == 4 and n_tbb == 2, "schedule hardcoded for S=2048"
            qk = emit_qk_chain
            emit_qk_chain(0, 0, True)    # k(dq0, tbb0)
            emit_qk_chain(1, 0, False)   # q(dq0, tbb1)
            emit_qk_chain(1, 0, True)    # k(dq0, tbb1)
            pairs = [(3, 0), (3, 1), (3, 2), (3, 3),
                     (2, 0), (2, 1), (2, 2), (2, 3),
                     (1, 0), (1, 1), (1, 2), (1, 3),
                     (0, 0), (0, 1), (0, 2), (0, 3)]
            # extra work spliced into each emit_pair call (call i covers
            # cur=pairs[i-1], nxt=pairs[i]; call 0 is the priming call).
            # nxt's scores (emitted from step 0) read kt tbb0 + qt tbb1, so
            # those chains MUST be spliced before step 0 (the tile
            # framework tracks write->read deps in emission order);
            # kt tbb1 is first read at step 8.
            splices = {
                0: {t: [lambda t=t: emit_v_chain(t)] for t in range(4)},
                1: {0: [lambda: qk(0, 1, True), lambda: qk(1, 1, False)],
                    2: [lambda: qk(1, 1, True)]},
                2: {0: [lambda: qk(0, 2, True), lambda: qk(1, 2, False)],
                    2: [lambda: qk(1, 2, True)]},
                3: {0: [lambda: qk(0, 3, True), lambda: qk(1, 3, False)],
                    2: [lambda: qk(1, 3, True)]},
                4: {0: [lambda: qk(0, 0, False)]},
                5: {0: [lambda: qk(0, 1, False)]},
                6: {0: [lambda: qk(0, 2, False)]},
                7: {0: [lambda: qk(0, 3, False)]},
            }
            for t in range(4, 16):
                splices[1].setdefault(t - 1, []).append(
                    lambda t=t: emit_v_chain(t))
            done_count = {qb: 0 for qb in range(n_qb)}
            pT_next = {}
            emit_pair(None, pairs[0], None, pT_next, extra=splices.get(0))
            ao_by_qb = {qb: [] for qb in range(n_qb)}
            oproj_queue = []
            for i, (qb, hp) in enumerate(pairs):
                pT_cur, pT_next = pT_next, {}
                nxt = pairs[i + 1] if i + 1 < len(pairs) else None
                if oproj_queue and oproj_queue[0][1] <= i:
                    oqb, _ = oproj_queue.pop(0)
                    emit_oproj(oqb, ao_by_qb.pop(oqb))
                ao_by_qb[qb].append(emit_pair(
                    (qb, hp), nxt, pT_cur, pT_next, extra=splices.get(i + 1)))
                done_count[qb] += 1
                if done_count[qb] == HH // 2:
                    oproj_queue.append((qb, i + 3))
            for oqb, _ in oproj_queue:
                emit_oproj(oqb, ao_by_qb.pop(oqb))

    nc.compile()
    return nc


_NC_CACHE = {}


def _get_nc(s=S):
    if s not in _NC_CACHE:
        _NC_CACHE[s] = build_nc(s)
    return _NC_CACHE[s]


def make_in_maps(x, w_q, w_k, w_v, w_o, s=S):
    """Host-side sharding: returns the 8 per-core input maps."""
    x = np.ascontiguousarray(np.asarray(x, dtype=np.float32))
    w_q = np.asarray(w_q, dtype=np.float32)
    w_k = np.asarray(w_k, dtype=np.float32)
    w_v = np.asarray(w_v, dtype=np.float32)
    w_o = np.asarray(w_o, dtype=np.float32)

    xTs = [np.ascontiguousarray(x[b].T.astype(np.float16)) for b in range(B)]
    wqTs = [np.ascontiguousarray(w_q[hg * HD:(hg + 1) * HD, :].T.astype(np.float16)) for hg in range(2)]
    wkTs = [np.ascontiguousarray(w_k[hg * HD:(hg + 1) * HD, :].T.astype(np.float16)) for hg in range(2)]
    wvTs = [np.ascontiguousarray(w_v[hg * HD:(hg + 1) * HD, :].T.astype(np.float16)) for hg in range(2)]
    woTs = [np.ascontiguousarray(w_o[:, hg * HD:(hg + 1) * HD].T.astype(np.float16)) for hg in range(2)]

    in_maps = []
    for c in range(N_CORES):
        b, hg = c // 2, c % 2
        in_maps.append({
            "xT": xTs[b], "wqT": wqTs[hg], "wkT": wkTs[hg],
            "wvT": wvTs[hg], "woT": woTs[hg],
        })
    return in_maps


def kernel(x, w_q, w_k, w_v, w_o, b_o):
    nc = _get_nc(S)
    in_maps = make_in_maps(x, w_q, w_k, w_v, w_o, s=S)
    res = run_bass_kernel_spmd(nc, in_maps, core_ids=list(range(N_CORES)))
    b_o = np.asarray(b_o, dtype=np.float32)
    outp = np.empty((B, S, D), dtype=np.float32)
    for b in range(B):
        outp[b] = (res.results[2 * b]["out"].astype(np.float32)
                   + res.results[2 * b + 1]["out"].astype(np.float32) + b_o)
    return outp


# revision 24
# speedup vs baseline: 1.0252x; 1.0111x over previous
"""Multi-head causal attention (B=4, S=2048, D=1024, H=16) on 8 TRN2 NeuronCores.

Sharding: core c handles batch b = c//2 and head-group hg = c%2 (8 heads each).
Each core computes Q/K/V projections for its (batch, head-group), causal
attention, and a partial output projection over its 512 head-dims.  The host
sums the two partials per batch and adds b_o.  No collectives.

Device-side layout choices:
  - x is passed transposed (xT [D, S]), loaded once into SBUF and reused by
    all projection chains.
  - Q and K are produced transposed (QT/KT [dq, S]); scores are computed
    transposed (S^T [kpos, q]).  K^T tiles keep the natural head-pair packing
    (head 2d in rows 0-63, head 2d+1 in rows 64-127) and the two heads'
    score matmuls run CONCURRENTLY on the PE via row tiling (64-row
    contraction each, tile_position (0,0) / (64,0)).
  - Scores for one kt tile and both heads land in one 2-bank [128, 1024]
    PSUM tile, so a single ACT exp covers both heads (halves ACT instruction
    overhead, the co-bottleneck).
  - No max-subtraction in softmax: scaled scores are ~N(0,1), exp is safe.
  - AV matmuls trim the below-diagonal (always-zero) query range instead of
    memsetting P; only the 128-wide diagonal square needs affine_select.
  - U (64 value dims + 64 denominator-broadcast rows, via ones columns in V)
    is copied out of PSUM to SBUF in fp16 right away so only 2 PSUM banks of
    accumulators are needed; 1/l = exp(-ln(l)) runs on the SBUF copy off the
    critical path (ln and exp share one ACT table set -> no table reloads).
"""

import sys
import os

sys.path.insert(0, "/opt/trn_rl_repo")

import numpy as np

import concourse.bacc as bacc
import concourse.mybir as mybir
import concourse.tile as tile
from concourse.bass_utils import run_bass_kernel_spmd

# The ACT table-load pass resolves each activation to the first table set
# containing it, which puts Exp (exp_and_others) and Ln
# (natural_log_exp_and_others) in different sets and reloads tables at every
# softmax normalization.  Restrict Exp/Ln to the one set that holds both so
# the whole kernel runs off a single table load.
_orig_get_tables = bacc.get_activation_tables


def _patched_tables(arch):
    t = _orig_get_tables(arch)
    for name, fns in t.items():
        if name != "natural_log_exp_and_others":
            fns.discard(mybir.ActivationFunctionType.Exp)
            fns.discard(mybir.ActivationFunctionType.Ln)
    return t


bacc.get_activation_tables = _patched_tables

B, S, D, H = 4, 2048, 1024, 16
DK = D // H          # 64
HH = H // 2          # 8 heads per core
HD = HH * DK         # 512 head-dims per core
N_CORES = 8

F32 = mybir.dt.float32
F16 = mybir.dt.float16

SCALE = 1.0 / np.sqrt(DK)


def build_nc(s=S):
    """Build the per-core SPMD program.  `s` is the sequence length (tunable
    for small-scale simulation; must be a multiple of 1024)."""
    assert s % 1024 == 0
    n_qb = s // 512          # 512-wide q blocks
    n_t128 = s // 128        # 128-wide token tiles
    n_tbb = s // 1024        # 1024-wide token blocks (projection chains)
    n_dt = D // 128          # din tiles (8)

    nc = bacc.Bacc("TRN2", target_bir_lowering=False, debug=False,
                   num_devices=N_CORES)

    xT = nc.dram_tensor("xT", [D, s], F16, kind="ExternalInput")
    wqT = nc.dram_tensor("wqT", [D, HD], F16, kind="ExternalInput")
    wkT = nc.dram_tensor("wkT", [D, HD], F16, kind="ExternalInput")
    wvT = nc.dram_tensor("wvT", [D, HD], F16, kind="ExternalInput")
    woT = nc.dram_tensor("woT", [HD, D], F16, kind="ExternalInput")
    out = nc.dram_tensor("out", [s, D], F16, kind="ExternalOutput")

    with tile.TileContext(nc) as tc:
        with tc.tile_pool(name="persist", bufs=1) as persist, \
             tc.tile_pool(name="pT", bufs=20) as pT_pool, \
             tc.tile_pool(name="usb", bufs=3) as usb_pool, \
             tc.tile_pool(name="rb", bufs=2) as rb_pool, \
             tc.tile_pool(name="aoT", bufs=12) as aoT_pool, \
             tc.tile_pool(name="outsb", bufs=2) as out_pool, \
             tc.tile_pool(name="bigp", bufs=3, space="PSUM") as bigp, \
             tc.tile_pool(name="upsum", bufs=2, space="PSUM") as upsum:

            # Persistent SBUF arrays (live for the whole kernel).
            # Q^T / K^T per head-pair: head 2d in rows 0-63, 2d+1 in 64-127.
            qt_sb = [persist.tile([128, s], F16, tag=f"qt{d}", name=f"qt{d}") for d in range(HD // 128)]
            kt_sb = [persist.tile([128, s], F16, tag=f"kt{d}", name=f"kt{d}") for d in range(HD // 128)]
            # V tiles hold [t, head, 2*dk]: cols 0-63 are V, cols 64-127 are
            # 1.0.  As the AV stationary this makes the matmul emit U^T on
            # psum rows 0-63 and the softmax denominator on rows 64-127.
            v_sb = [persist.tile([128, HH, 2 * DK], F16, tag=f"v{t}", name=f"v{t}") for t in range(n_t128)]
            wo_sb = [persist.tile([128, D], F16, tag=f"wo{d}", name=f"wo{d}") for d in range(HD // 128)]
            wv_sb = [persist.tile([128, HD], F16, tag=f"wv{i}", name=f"wv{i}") for i in range(n_dt)]
            # x resident: [tbb][i] -> [128, 1024] (din tile i, token block tbb)
            x_sb = [[persist.tile([128, 1024], F16, tag=f"x{tbb}_{i}", name=f"x{tbb}_{i}")
                     for i in range(n_dt)] for tbb in range(n_tbb)]
            # Q/K weights in [128, 256] column slices (dq pairs) so the first
            # chain is gated on only 8 small DMAs.
            w_tiles = {}
            for wkey in ("q", "k"):
                for i in range(n_dt):
                    for dqp in range(HD // 256):
                        w_tiles[(wkey, i, dqp)] = persist.tile(
                            [128, 256], F16, tag=f"w{wkey}{i}_{dqp}",
                            name=f"w{wkey}{i}_{dqp}")

            # Ones columns of the V tiles (written once; V chains only write
            # cols 0-63).  On gpsimd to keep the DVE free.
            for t in range(n_t128):
                nc.gpsimd.memset(v_sb[t][:, :, DK:2 * DK], 1.0)

            # Warmup matmuls on scratch data: keep the PE busy through the
            # initial DMA lead-in so the HAM clock-gate is warm (2.4 GHz)
            # when real work starts.  Scratch memset on the (idle) DVE so
            # the warmup isn't queued behind the gpsimd memsets.
            scratch = persist.tile([128, 512], F16, tag="scratch", name="scratch")
            nc.vector.memset(scratch[:], 0.0)
            wm = bigp.tile([128, 1024], F32, tag="big", name="wm")
            for _ in range(28):
                nc.tensor.matmul(wm[:, 0:512], lhsT=scratch[:, 0:128],
                                 rhs=scratch[:], start=True, stop=True)

            # DMAs in first-use order for the descending-block schedule:
            # k(dq0,tbb0) gates the first chain, then q(dq0,tbb1) needs wq
            # dqp0 + x tbb1, then wv (V chains start during the priming
            # pair), then the dq2-3 weights, then wo (out-proj is late).
            for i in range(n_dt):
                nc.sync.dma_start(
                    out=w_tiles[("k", i, 0)][:],
                    in_=wkT[i * 128:(i + 1) * 128, 0:256])
                nc.sync.dma_start(
                    out=x_sb[0][i][:], in_=xT[i * 128:(i + 1) * 128, 0:1024])
            for i in range(n_dt):
                nc.sync.dma_start(
                    out=w_tiles[("q", i, 0)][:],
                    in_=wqT[i * 128:(i + 1) * 128, 0:256])
                if n_tbb > 1:
                    nc.sync.dma_start(
                        out=x_sb[1][i][:],
                        in_=xT[i * 128:(i + 1) * 128, 1024:2048])
            for i in range(n_dt):
                nc.sync.dma_start(out=wv_sb[i][:], in_=wvT[i * 128:(i + 1) * 128, :])
            for dqp in range(1, HD // 256):
                for wkey, wdram in (("k", wkT), ("q", wqT)):
                    for i in range(n_dt):
                        nc.sync.dma_start(
                            out=w_tiles[(wkey, i, dqp)][:],
                            in_=wdram[i * 128:(i + 1) * 128,
                                      dqp * 256:(dqp + 1) * 256])
            for d in range(HD // 128):
                nc.sync.dma_start(out=wo_sb[d][:], in_=woT[d * 128:(d + 1) * 128, :])

            def emit_qk_chain(tbb, dq, is_k):
                """Q^T or K^T projection for one 1024-token block and one
                head-pair dq, off the resident x tiles."""
                wkey = "k" if is_k else "q"
                ps = bigp.tile([128, 1024], F32, tag="big", name="qk")
                # i outer / half inner: consecutive matmuls share the same
                # stationary weights.
                for i in range(n_dt):
                    w = w_tiles[(wkey, i, dq // 2)][:, (dq % 2) * 128:(dq % 2 + 1) * 128]
                    for half in range(2):
                        nc.tensor.matmul(
                            ps[:, half * 512:(half + 1) * 512],
                            lhsT=w,
                            rhs=x_sb[tbb][i][:, half * 512:(half + 1) * 512],
                            start=(i == 0), stop=(i == n_dt - 1),
                        )
                dst = kt_sb[dq] if is_k else qt_sb[dq]
                nc.vector.tensor_copy(
                    out=dst[:, tbb * 1024:(tbb + 1) * 1024], in_=ps[:])

            def emit_v_chain(tb):
                """V projection for one 128-token tile, spliced into the
                attention stream just before the q-block that needs it."""
                vp = bigp.tile([128, 1024], F32, tag="big", name="vp")
                for i in range(n_dt):
                    xs = x_sb[tb // 8][i][:, (tb % 8) * 128:(tb % 8 + 1) * 128]
                    nc.tensor.matmul(
                        vp[:, 0:512], lhsT=xs, rhs=wv_sb[i][:],
                        start=(i == 0), stop=(i == n_dt - 1),
                    )
                nc.vector.tensor_copy(
                    out=v_sb[tb][:, :, 0:DK],
                    in_=vp[:, 0:512].rearrange("p (h k) -> p h k", h=HH))

            def emit_score_kt(qb, hp, kt, pT):
                """Scores + exp for one kt tile, BOTH heads of the pair:
                row-tiled matmuls (64-contraction each) into one 2-bank psum
                tile, one exp over both."""
                lo = max(kt - 4 * qb, 0) * 128
                sp = bigp.tile([128, 1024], F32, tag="big", name="sp")
                for hh in (0, 1):
                    nc.tensor.matmul(
                        sp[:, hh * 512 + lo:(hh + 1) * 512],
                        lhsT=kt_sb[hp][hh * 64:(hh + 1) * 64,
                                       kt * 128:(kt + 1) * 128],
                        rhs=qt_sb[hp][hh * 64:(hh + 1) * 64,
                                      qb * 512 + lo:(qb + 1) * 512],
                        start=True, stop=True,
                    )
                p = pT_pool.tile([128, 1024], F16, tag="p", name="p")
                nc.scalar.activation(
                    out=p[:, lo:1024], in_=sp[:, lo:1024],
                    func=mybir.ActivationFunctionType.Exp,
                    scale=float(SCALE))
                if kt >= 4 * qb:
                    # zero strict-upper (kpos > q) region of the 128-wide
                    # diagonal square, per head
                    for hh in (0, 1):
                        nc.gpsimd.affine_select(
                            out=p[:, hh * 512 + lo:hh * 512 + lo + 128],
                            in_=p[:, hh * 512 + lo:hh * 512 + lo + 128],
                            compare_op=mybir.AluOpType.is_ge,
                            fill=0.0, base=0, channel_multiplier=-1,
                            pattern=[[1, 128]])
                pT[kt] = (p, lo)

            def emit_pair(cur, nxt, pT_cur, pT_next, extra=None):
                """Interleave next pair's scores with current pair's AV
                chains at kt granularity: the PE gets AV matmuls to run
                while the ACT engine works through the scores' exps.
                `extra` maps a kt step to closures (projection/V chains)
                spliced in at that step."""
                nkt_cur = 4 * cur[0] + 4 if cur else 0
                nkt_nxt = 4 * nxt[0] + 4 if nxt else 0
                u = {}
                if cur:
                    for hh in (0, 1):
                        u[hh] = upsum.tile([128, 512], F32, tag="u", name="u")
                nsteps = max(nkt_cur, nkt_nxt)
                for kt in range(nsteps):
                    for work in (extra or {}).get(kt, ()):
                        work()
                    if kt < nkt_nxt:
                        emit_score_kt(nxt[0], nxt[1], kt, pT_next)
                    if kt < nkt_cur:
                        p, lo = pT_cur[kt]
                        for hh in (0, 1):
                            nc.tensor.matmul(
                                u[hh][:, lo:512],
                                lhsT=v_sb[kt][:, 2 * cur[1] + hh, :],
                                rhs=p[:, hh * 512 + lo:(hh + 1) * 512],
                                start=(kt == 0), stop=(kt == nkt_cur - 1),
                            )
                for work in (extra or {}).get(nsteps, ()):
                    work()
                if not cur:
                    return None
                # Evacuate U to SBUF fp16 (rows 64-127: PSUM reads can be
                # partition-shifted, SBUF-SBUF operands can't) and take
                # ln of the denominators straight from PSUM; both free the
                # psum bank quickly for the next pair.
                usb = usb_pool.tile([128, 1024], F16, tag="usb", name="usb")
                rb = rb_pool.tile([128, 1024], F32, tag="rb", name="rb")
                for hh in (0, 1):
                    with tc.high_priority(offset=300):
                        nc.vector.tensor_copy(
                            out=usb[64:128, hh * 512:(hh + 1) * 512],
                            in_=u[hh][0:64, :])
                    nc.scalar.activation(
                        out=rb[64:128, hh * 512:(hh + 1) * 512],
                        in_=u[hh][64:128, :],
                        func=mybir.ActivationFunctionType.Ln)
                # 1/l = exp(-ln(l)): ln and exp share one ACT table set.
                nc.scalar.activation(
                    out=rb[64:128, :], in_=rb[64:128, :],
                    func=mybir.ActivationFunctionType.Exp, scale=-1.0)
                ao = aoT_pool.tile([128, 512], F16, tag="aoT", name="aoT")
                for hh in (0, 1):
                    nc.vector.tensor_mul(
                        out=ao[hh * 64:(hh + 1) * 64, :],
                        in0=usb[64:128, hh * 512:(hh + 1) * 512],
                        in1=rb[64:128, hh * 512:(hh + 1) * 512])
                return ao

            def emit_oproj(qb, ao_pairs):
                for qt_l in range(4):
                    qt = 4 * qb + qt_l
                    op = bigp.tile([128, 1024], F32, tag="big", name="op")
                    # hp outer / half inner: consecutive matmuls share the
                    # same stationary ao tile.
                    for hp in range(HH // 2):
                        for half in range(2):
                            nc.tensor.matmul(
                                op[:, half * 512:(half + 1) * 512],
                                lhsT=ao_pairs[hp][:, qt_l * 128:(qt_l + 1) * 128],
                                rhs=wo_sb[hp][:, half * 512:(half + 1) * 512],
                                start=(hp == 0), stop=(hp == 3),
                            )
                    osb = out_pool.tile([128, D], F16, tag="osb", name="osb")
                    nc.vector.tensor_copy(out=osb[:], in_=op[:])
                    nc.sync.dma_start(
                        out=out[qt * 128:(qt + 1) * 128, :], in_=osb[:])

            # Schedule: pairs run in DESCENDING q-block order.  Late blocks
            # are exp(ACT)-heavy relative to their matmul work, early blocks
            # the reverse, and the projection chains are pure PE -- so the
            # heaviest pairs start as early as possible (gated by only 3
            # chains) and all remaining chain/V work is spliced into them.
            # This keeps the ACT engine fed from ~10us on and keeps the PE
            # dense enough that the HAM clock-gate stays at full rate.
            assert n_qb == 4 and n_tbb == 2, "schedule hardcoded for S=2048"
            qk = emit_qk_chain
            emit_qk_chain(0, 0, True)    # k(dq0, tbb0)
            emit_qk_chain(1, 0, False)   # q(dq0, tbb1)
            emit_qk_chain(1, 0, True)    # k(dq0, tbb1)
            pairs = [(3, 0), (3, 1), (3, 2), (3, 3),
                     (2, 0), (2, 1), (2, 2), (2, 3),
                     (1, 0), (1, 1), (1, 2), (1, 3),
                     (0, 0), (0, 1), (0, 2), (0, 3)]
            # extra work spliced into each emit_pair call (call i covers
            # cur=pairs[i-1], nxt=pairs[i]; call 0 is the priming call).
            # nxt's scores (emitted from step 0) read kt tbb0 + qt tbb1, so
            # those chains MUST be spliced before step 0 (the tile
            # framework tracks write->read deps in emission order);
            # kt tbb1 is first read at step 8.
            splices = {
                0: {t: [lambda t=t: emit_v_chain(t)] for t in range(4)},
                1: {0: [lambda: qk(0, 1, True), lambda: qk(1, 1, False)],
                    2: [lambda: qk(1, 1, True)]},
                2: {0: [lambda: qk(0, 2, True), lambda: qk(1, 2, False)],
                    2: [lambda: qk(1, 2, True)]},
                3: {0: [lambda: qk(0, 3, True), lambda: qk(1, 3, False)],
                    2: [lambda: qk(1, 3, True)]},
                4: {0: [lambda: qk(0, 0, False)]},
                5: {0: [lambda: qk(0, 1, False)]},
                6: {0: [lambda: qk(0, 2, False)]},
                7: {0: [lambda: qk(0, 3, False)]},
            }
            for t in range(4, 16):
                splices[1].setdefault(t - 1, []).append(
                    lambda t=t: emit_v_chain(t))
            done_count = {qb: 0 for qb in range(n_qb)}
            pT_next = {}
            emit_pair(None, pairs[0], None, pT_next, extra=splices.get(0))
            ao_by_qb = {qb: [] for qb in range(n_qb)}
            oproj_queue = []
            for i, (qb, hp) in enumerate(pairs):
                pT_cur, pT_next = pT_next, {}
                nxt = pairs[i + 1] if i + 1 < len(pairs) else None
                if oproj_queue and oproj_queue[0][1] <= i:
                    oqb, _ = oproj_queue.pop(0)
                    emit_oproj(oqb, ao_by_qb.pop(oqb))
                ao_by_qb[qb].append(emit_pair(
                    (qb, hp), nxt, pT_cur, pT_next, extra=splices.get(i + 1)))
                done_count[qb] += 1
                if done_count[qb] == HH // 2:
                    oproj_queue.append((qb, i + 3))
            for oqb, _ in oproj_queue:
                emit_oproj(oqb, ao_by_qb.pop(oqb))

    nc.compile()
    return nc


_NC_CACHE = {}


def _get_nc(s=S):
    if s not in _NC_CACHE:
        _NC_CACHE[s] = build_nc(s)
    return _NC_CACHE[s]


def make_in_maps(x, w_q, w_k, w_v, w_o, s=S):
    """Host-side sharding: returns the 8 per-core input maps."""
    x = np.ascontiguousarray(np.asarray(x, dtype=np.float32))
    w_q = np.asarray(w_q, dtype=np.float32)
    w_k = np.asarray(w_k, dtype=np.float32)
    w_v = np.asarray(w_v, dtype=np.float32)
    w_o = np.asarray(w_o, dtype=np.float32)

    xTs = [np.ascontiguousarray(x[b].T.astype(np.float16)) for b in range(B)]
    wqTs = [np.ascontiguousarray(w_q[hg * HD:(hg + 1) * HD, :].T.astype(np.float16)) for hg in range(2)]
    wkTs = [np.ascontiguousarray(w_k[hg * HD:(hg + 1) * HD, :].T.astype(np.float16)) for hg in range(2)]
    wvTs = [np.ascontiguousarray(w_v[hg * HD:(hg + 1) * HD, :].T.astype(np.float16)) for hg in range(2)]
    woTs = [np.ascontiguousarray(w_o[:, hg * HD:(hg + 1) * HD].T.astype(np.float16)) for hg in range(2)]

    in_maps = []
    for c in range(N_CORES):
        b, hg = c // 2, c % 2
        in_maps.append({
            "xT": xTs[b], "wqT": wqTs[hg], "wkT": wkTs[hg],
            "wvT": wvTs[hg], "woT": woTs[hg],
        })
    return in_maps


def kernel(x, w_q, w_k, w_v, w_o, b_o):
    nc = _get_nc(S)
    in_maps = make_in_maps(x, w_q, w_k, w_v, w_o, s=S)
    res = run_bass_kernel_spmd(nc, in_maps, core_ids=list(range(N_CORES)))
    b_o = np.asarray(b_o, dtype=np.float32)
    outp = np.empty((B, S, D), dtype=np.float32)
    for b in range(B):
        outp[b] = (res.results[2 * b]["out"].astype(np.float32)
                   + res.results[2 * b + 1]["out"].astype(np.float32) + b_o)
    return outp
